# revision 4
# baseline (speedup 1.0000x reference)
"""MoE low-rank conv kernel for Trainium2 (8 NeuronCores, expert-dispatch).

Reference computation (B=8, C=512, H=W=64, E=8, R=64, TOP_K=2):
  expert_e(x) = conv1x1(conv3x3(x, wA[e]) + bA[e], wB[e]) + bB[e]
  logits = meanpool(conv1x1(x, wG) + bG);  probs = softmax(logits)
  gw, idx = top_k(probs, 2)
  out[b] = sum_k gw[b,k] * expert_{idx[b,k]}(x[b]) * x[b]
  returns (out, gw[:, :, None, None, None])

Strategy: gating/top-k is tiny -> computed on host exactly. Only the
2 selected experts per sample are computed on device (the other 6 experts
never influence the output). One NeuronCore per batch sample; the two
selected experts are packed side by side into the 128-wide PE array
(conv3x3 lhsT: K=128 input-channel chunk, M = 2 experts x 64 ranks) so all
matmuls run the full 128x128 array. conv3x3 is computed as 9 shifted
accumulating matmuls over a zero-padded SBUF image. The 1x1 conv of both
experts *and* the gate-weighted sum collapse into a single K=128 matmul per
output tile by pre-scaling wB with the gate weights. Biases ride along on
the ScalarE psum->SBUF evacuations. Matmuls use float32r (FP22) which runs
at full PE rate with ~1e-4 relative precision.
"""

import numpy as np
from contextlib import ExitStack

import jax
from jax.sharding import Mesh, PartitionSpec

import concourse.bass as bass
import concourse.tile as tile
from concourse import bacc, mybir

B, C, H, W = 8, 512, 64, 64
E, R, TOP_K = 8, 64, 2
NCH = C // 128          # 4 input/output channel chunks
NBLK = H // 8           # 8 row-blocks of 8 rows (8*64 = 512 cols = 1 psum bank)
F32 = mybir.dt.float32
F32R = mybir.dt.float32r

_CACHE: dict = {}


def _build_nc():
    nc = bacc.Bacc("TRN2", target_bir_lowering=False, debug=False)
    x_d = nc.dram_tensor("x", [C, H + 2, W + 2], F32R, kind="ExternalInput").ap()
    wa_d = nc.dram_tensor("wa", [128, 9 * NCH * 128], F32R, kind="ExternalInput").ap()
    wb_d = nc.dram_tensor("wb", [128, C], F32R, kind="ExternalInput").ap()
    cc_d = nc.dram_tensor("cc", [128, NCH], F32, kind="ExternalInput").ap()
    ba_d = nc.dram_tensor("ba", [128, 1], F32, kind="ExternalInput").ap()
    out_d = nc.dram_tensor("out", [C, H, W], F32, kind="ExternalOutput").ap()

    with tile.TileContext(nc) as tc:
        with ExitStack() as ctx:
            singles = ctx.enter_context(tc.tile_pool(name="singles", bufs=1))
            work = ctx.enter_context(tc.tile_pool(name="work", bufs=3))
            psa = ctx.enter_context(tc.tile_pool(name="psa", bufs=2, space="PSUM"))
            psb = ctx.enter_context(tc.tile_pool(name="psb", bufs=2, space="PSUM"))

            # ---- resident inputs ----
            wa_t = singles.tile([128, 9 * NCH * 128], F32R)
            nc.sync.dma_start(wa_t[:], wa_d[:])
            wb_t = singles.tile([128, C], F32R)
            nc.sync.dma_start(wb_t[:], wb_d[:])
            cc_t = singles.tile([128, NCH], F32)
            nc.sync.dma_start(cc_t[:], cc_d[:])
            ba_t = singles.tile([128, 1], F32)
            nc.sync.dma_start(ba_t[:], ba_d[:])

            # padded input image: [chunk, 66, 66] per partition, zero border
            xp = singles.tile([128, NCH, H + 2, W + 2], F32R)
            for k in range(NCH):
                nc.sync.dma_start(
                    xp[:, k, :, :],
                    x_d[k * 128 : (k + 1) * 128, :, :],
                )

            # conv3x3 outputs (+bias) for both experts: rank r of expert i at
            # partition i*64+r
            hh = singles.tile([128, H * W], F32R)

            # ---- conv3x3 as 9 shifted matmuls, both experts packed in M ----
            for blk in range(NBLK):
                h0 = blk * 8
                ph = psa.tile([128, 8 * W], F32)
                first, last = (0, 0), (2, 2, NCH - 1)
                for dy in range(3):
                    for dx in range(3):
                        for k in range(NCH):
                            o = dy * 3 + dx
                            lhsT = wa_t[:, (o * NCH + k) * 128 : (o * NCH + k + 1) * 128]
                            rhs = xp[:, k, h0 + dy : h0 + dy + 8, dx : dx + W]
                            nc.tensor.matmul(
                                ph[:],
                                lhsT,
                                rhs,
                                start=(dy, dx, k) == (0, 0, 0),
                                stop=(dy, dx, k) == last,
                            )
                nc.scalar.activation(
                    out=hh[:, h0 * W : (h0 + 8) * W],
                    in_=ph[:],
                    func=mybir.ActivationFunctionType.Identity,
                    bias=ba_t[:, 0:1],
                )

            # ---- 1x1 conv + gate-weighted combine + (*x) + store ----
            for m in range(NCH):
                for blk in range(NBLK):
                    h0 = blk * 8
                    py = psb.tile([128, 8, W], F32)
                    nc.tensor.matmul(
                        py[:],
                        wb_t[:, m * 128 : (m + 1) * 128],
                        hh[:, h0 * W : (h0 + 8) * W],
                        start=True,
                        stop=True,
                    )
                    yb = work.tile([128, 8, W], F32)
                    nc.scalar.activation(
                        out=yb[:],
                        in_=py[:],
                        func=mybir.ActivationFunctionType.Identity,
                        bias=cc_t[:, m : m + 1],
                    )
                    ot = work.tile([128, 8, W], F32)
                    nc.vector.tensor_mul(
                        ot[:], yb[:], xp[:, m, h0 + 1 : h0 + 9, 1 : W + 1].bitcast(F32)
                    )
                    nc.sync.dma_start(
                        out_d[m * 128 : (m + 1) * 128, h0 : h0 + 8, :], ot[:]
                    )
    nc.compile()
    return nc


def _get_runner():
    """Compile once; return a function(in_maps) -> list of per-core out dicts."""
    if "runner" in _CACHE:
        return _CACHE["runner"]

    from concourse import bass2jax

    nc = _build_nc()
    bass2jax.install_neuronx_cc_hook()

    partition_name = nc.partition_id_tensor.name if nc.partition_id_tensor else None
    in_names, out_names, out_avals, zero_shapes = [], [], [], []
    for alloc in nc.m.functions[0].allocations:
        if not isinstance(alloc, mybir.MemoryLocationSet):
            continue
        name = alloc.memorylocations[0].name
        if alloc.kind == "ExternalInput":
            if name != partition_name:
                in_names.append(name)
        elif alloc.kind == "ExternalOutput":
            out_names.append(name)
            shape = tuple(alloc.tensor_shape)
            dtype = mybir.dt.np(alloc.dtype)
            out_avals.append(jax.core.ShapedArray(shape, dtype))
            zero_shapes.append((shape, dtype))
    n_params = len(in_names)
    n_outs = len(out_names)
    all_names = in_names + out_names
    if partition_name is not None:
        all_names = all_names + [partition_name]
    donate = tuple(range(n_params, n_params + n_outs))

    def _body(*args):
        operands = list(args)
        if partition_name is not None:
            operands.append(bass2jax.partition_id_tensor())
        outs = bass2jax._bass_exec_p.bind(
            *operands,
            out_avals=tuple(out_avals),
            in_names=tuple(all_names),
            out_names=tuple(out_names),
            lowering_input_output_aliases=(),
            sim_require_finite=True,
            sim_require_nnan=True,
            nc=nc,
        )
        return tuple(outs)

    devices = jax.devices()[:B]
    mesh = Mesh(np.asarray(devices), ("core",))
    sharded = jax.jit(
        bass2jax.shard_map(
            _body,
            mesh=mesh,
            in_specs=(PartitionSpec("core"),) * (n_params + n_outs),
            out_specs=(PartitionSpec("core"),) * n_outs,
            check_rep=False,
        ),
        donate_argnums=donate,
        keep_unused=True,
    )

    def run(in_maps):
        concat_in = [
            np.concatenate([np.asarray(m[name]) for m in in_maps], axis=0)
            for name in in_names
        ]
        concat_zeros = [
            np.zeros((B * s[0], *s[1:]), d) for (s, d) in zero_shapes
        ]
        out_arrs = sharded(*concat_in, *concat_zeros)
        return [
            {
                name: np.asarray(out_arrs[i]).reshape(B, *out_avals[i].shape)[c]
                for i, name in enumerate(out_names)
            }
            for c in range(B)
        ]

    _CACHE["runner"] = run
    return run


def _gating(x, wG, bG):
    """Exact host-side gating: logits -> softmax -> top-2 (matches jax)."""
    xbar = x.astype(np.float64).mean(axis=(2, 3))            # [B, C]
    logits = xbar @ wG[:, :, 0, 0].astype(np.float64).T + bG.astype(np.float64)
    logits -= logits.max(axis=1, keepdims=True)
    p = np.exp(logits)
    probs = p / p.sum(axis=1, keepdims=True)
    idx = np.argsort(-probs, axis=1, kind="stable")[:, :TOP_K]
    gw = np.take_along_axis(probs, idx, axis=1)
    return gw.astype(np.float32), idx


def _make_in_maps(x, wA, bA, wB, bB, gw, idx):
    in_maps = []
    for b in range(B):
        e0, e1 = int(idx[b, 0]), int(idx[b, 1])
        g0, g1 = float(gw[b, 0]), float(gw[b, 1])
        # conv3x3 lhsT: [tap, c-chunk, c%128, m] with m = expert*64 + rank
        wa2 = wA[[e0, e1]]                                   # [2, R, C, 3, 3]
        t = wa2.transpose(3, 4, 2, 0, 1).reshape(9, C, 128)   # [o, c, m]
        wa_host = np.ascontiguousarray(
            t.reshape(9, NCH, 128, 128).transpose(2, 0, 1, 3).reshape(128, -1)
        ).astype(np.float32)
        # 1x1 conv lhsT with gates folded in: rows = [g0*wB[e0].T ; g1*wB[e1].T]
        wb_host = np.concatenate(
            [g0 * wB[e0, :, :, 0, 0].T, g1 * wB[e1, :, :, 0, 0].T], axis=0
        ).astype(np.float32)
        wb_host = np.ascontiguousarray(wb_host)
        cc = (g0 * bB[e0] + g1 * bB[e1]).astype(np.float32)   # [C]
        cc_host = np.ascontiguousarray(cc.reshape(NCH, 128).T)
        ba_host = np.concatenate([bA[e0], bA[e1]]).astype(np.float32).reshape(128, 1)
        in_maps.append(
            {
                "x": np.pad(x[b], ((0, 0), (1, 1), (1, 1))),
                "wa": wa_host,
                "wb": wb_host,
                "cc": np.ascontiguousarray(cc_host),
                "ba": np.ascontiguousarray(ba_host),
            }
        )
    return in_maps


def kernel(x, wA, bA, wB, bB, wG, bG):
    x = np.asarray(x, dtype=np.float32)
    wA = np.asarray(wA, dtype=np.float32)
    bA = np.asarray(bA, dtype=np.float32)
    wB = np.asarray(wB, dtype=np.float32)
    bB = np.asarray(bB, dtype=np.float32)
    wG = np.asarray(wG, dtype=np.float32)
    bG = np.asarray(bG, dtype=np.float32)

    gw, idx = _gating(x, wG, bG)
    in_maps = _make_in_maps(x, wA, bA, wB, bB, gw, idx)
    results = _get_runner()(in_maps)
    out = np.stack([results[b]["out"] for b in range(B)], axis=0)
    return out, gw[:, :, None, None, None]


# revision 8
# speedup vs baseline: 15214.4780x; 15214.4780x over previous
"""MoE low-rank conv kernel for Trainium2 (8 NeuronCores, expert-dispatch).

Reference computation (B=8, C=512, H=W=64, E=8, R=64, TOP_K=2):
  expert_e(x) = conv1x1(conv3x3(x, wA[e]) + bA[e], wB[e]) + bB[e]
  logits = meanpool(conv1x1(x, wG) + bG);  probs = softmax(logits)
  gw, idx = top_k(probs, 2)
  out[b] = sum_k gw[b,k] * expert_{idx[b,k]}(x[b]) * x[b]
  returns (out, gw[:, :, None, None, None])

Strategy: gating/top-k is tiny -> computed on host exactly. Only the
2 selected experts per sample are computed on device (the other 6 experts
never influence the output). One NeuronCore per batch sample; the two
selected experts are packed side by side into the 128-wide PE array
(conv3x3 lhsT: K=128 input-channel chunk, M = 2 experts x 64 ranks) so all
matmuls run the full 128x128 array. conv3x3 is computed as 9 shifted
accumulating matmuls over a zero-padded SBUF image. The 1x1 conv of both
experts *and* the gate-weighted sum collapse into a single K=128 matmul per
output tile by pre-scaling wB with the gate weights. Biases ride along on
the ScalarE psum->SBUF evacuations. Matmuls use float32r (FP22) which runs
at full PE rate with ~1e-4 relative precision.
"""

import numpy as np
from contextlib import ExitStack

import jax
from jax.sharding import Mesh, PartitionSpec

import concourse.bass as bass
import concourse.tile as tile
from concourse import bacc, mybir

B, C, H, W = 8, 512, 64, 64
E, R, TOP_K = 8, 64, 2
NCH = C // 128          # 4 input/output channel chunks
NBLK = H // 8           # 8 row-blocks of 8 rows (8*64 = 512 cols = 1 psum bank)
F32 = mybir.dt.float32
F32R = mybir.dt.float32r

_CACHE: dict = {}


def _build_nc(repeat=1):
    nc = bacc.Bacc("TRN2", target_bir_lowering=False, debug=False)
    x_d = nc.dram_tensor("x", [C, H + 2, W + 2], F32R, kind="ExternalInput").ap()
    wa_d = nc.dram_tensor("wa", [128, 9 * NCH * 128], F32R, kind="ExternalInput").ap()
    wb_d = nc.dram_tensor("wb", [128, C], F32R, kind="ExternalInput").ap()
    cc_d = nc.dram_tensor("cc", [128, NCH], F32, kind="ExternalInput").ap()
    ba_d = nc.dram_tensor("ba", [128, 1], F32, kind="ExternalInput").ap()
    out_d = nc.dram_tensor("out", [C, H, W], F32, kind="ExternalOutput").ap()

    with tile.TileContext(nc) as tc:
        with ExitStack() as ctx:
            singles = ctx.enter_context(tc.tile_pool(name="singles", bufs=1))
            work = ctx.enter_context(tc.tile_pool(name="work", bufs=3))
            psa = ctx.enter_context(tc.tile_pool(name="psa", bufs=2, space="PSUM"))
            psb = ctx.enter_context(tc.tile_pool(name="psb", bufs=3, space="PSUM"))

            # ---- resident inputs ----
            # DMA issue order = arrival order on the serial queue model, so
            # the pieces the first conv block needs are issued first:
            # x rows [0,10) of all chunks, tap-0 weights, bias; the rest of
            # x and the remaining taps stream in block-consumption order.
            wa_t = singles.tile([128, 9 * NCH * 128], F32R)
            xp = singles.tile([128, NCH, H + 2, W + 2], F32R)
            row_groups = [(0, 10)] + [(10 + 8 * i, 18 + 8 * i) for i in range(7)]
            for k in range(NCH):
                r0, r1 = row_groups[0]
                nc.sync.dma_start(
                    xp[:, k, r0:r1, :], x_d[k * 128 : (k + 1) * 128, r0:r1, :]
                )
            nc.sync.dma_start(wa_t[:, 0 : NCH * 128], wa_d[:, 0 : NCH * 128])
            ba_t = singles.tile([128, 1], F32)
            nc.sync.dma_start(ba_t[:], ba_d[:])
            for o in range(1, 9):
                c0, c1 = o * NCH * 128, (o + 1) * NCH * 128
                nc.sync.dma_start(wa_t[:, c0:c1], wa_d[:, c0:c1])
                if o < 8:
                    r0, r1 = row_groups[o]
                    for k in range(NCH):
                        nc.sync.dma_start(
                            xp[:, k, r0:r1, :],
                            x_d[k * 128 : (k + 1) * 128, r0:r1, :],
                        )
            wb_t = singles.tile([128, C], F32R)
            nc.sync.dma_start(wb_t[:], wb_d[:])
            cc_t = singles.tile([128, NCH], F32)
            nc.sync.dma_start(cc_t[:], cc_d[:])

            # conv3x3 outputs (+bias) for both experts: rank r of expert i at
            # partition i*64+r
            hh = singles.tile([128, H * W], F32R)

            def conv_a_block(blk):
                # ---- conv3x3: 9 shifted accumulating matmuls x 4 c-chunks ----
                h0 = blk * 8
                ph = psa.tile([128, 8 * W], F32)
                last = (2, 2, NCH - 1)
                for dy in range(3):
                    for dx in range(3):
                        for k in range(NCH):
                            o = dy * 3 + dx
                            lhsT = wa_t[:, (o * NCH + k) * 128 : (o * NCH + k + 1) * 128]
                            rhs = xp[:, k, h0 + dy : h0 + dy + 8, dx : dx + W]
                            nc.tensor.matmul(
                                ph[:],
                                lhsT,
                                rhs,
                                start=(dy, dx, k) == (0, 0, 0),
                                stop=(dy, dx, k) == last,
                            )
                nc.scalar.activation(
                    out=hh[:, h0 * W : (h0 + 8) * W],
                    in_=ph[:],
                    func=mybir.ActivationFunctionType.Identity,
                    bias=ba_t[:, 0:1],
                )

            def conv_b_block(blk):
                # ---- 1x1 conv + gate-weighted combine + (*x) + store ----
                h0 = blk * 8
                for m in range(NCH):
                    py = psb.tile([128, 8, W], F32)
                    nc.tensor.matmul(
                        py[:],
                        wb_t[:, m * 128 : (m + 1) * 128],
                        hh[:, h0 * W : (h0 + 8) * W],
                        start=True,
                        stop=True,
                    )
                    yb = work.tile([128, 8, W], F32)
                    nc.scalar.activation(
                        out=yb[:],
                        in_=py[:],
                        func=mybir.ActivationFunctionType.Identity,
                        bias=cc_t[:, m : m + 1],
                    )
                    ot = work.tile([128, 8, W], F32)
                    nc.vector.tensor_mul(
                        ot[:], yb[:], xp[:, m, h0 + 1 : h0 + 9, 1 : W + 1].bitcast(F32)
                    )
                    nc.sync.dma_start(
                        out_d[m * 128 : (m + 1) * 128, h0 : h0 + 8, :], ot[:]
                    )

            # repeat>1 builds a benchmarking NEFF that replays the body.
            # conv B of block k-1 is emitted between conv A blocks k and k+1
            # so the PE instruction stream never waits on ACT/DVE consumers.
            for _rep in range(repeat):
                for blk in range(NBLK):
                    conv_a_block(blk)
                    if blk >= 1:
                        conv_b_block(blk - 1)
                conv_b_block(NBLK - 1)
    nc.compile()
    return nc


def _get_runner(repeat=1):
    """Compile once; return a function(in_maps) -> list of per-core out dicts."""
    key = ("runner", repeat)
    if key in _CACHE:
        return _CACHE[key]

    from concourse import bass2jax

    nc = _build_nc(repeat)
    bass2jax.install_neuronx_cc_hook()

    partition_name = nc.partition_id_tensor.name if nc.partition_id_tensor else None
    in_names, out_names, out_avals, zero_shapes = [], [], [], []
    for alloc in nc.m.functions[0].allocations:
        if not isinstance(alloc, mybir.MemoryLocationSet):
            continue
        name = alloc.memorylocations[0].name
        if alloc.kind == "ExternalInput":
            if name != partition_name:
                in_names.append(name)
        elif alloc.kind == "ExternalOutput":
            out_names.append(name)
            shape = tuple(alloc.tensor_shape)
            dtype = mybir.dt.np(alloc.dtype)
            out_avals.append(jax.core.ShapedArray(shape, dtype))
            zero_shapes.append((shape, dtype))
    n_params = len(in_names)
    n_outs = len(out_names)
    all_names = in_names + out_names
    if partition_name is not None:
        all_names = all_names + [partition_name]
    del n_outs

    def _body(*args):
        operands = list(args)
        if partition_name is not None:
            operands.append(bass2jax.partition_id_tensor())
        outs = bass2jax._bass_exec_p.bind(
            *operands,
            out_avals=tuple(out_avals),
            in_names=tuple(all_names),
            out_names=tuple(out_names),
            lowering_input_output_aliases=(),
            sim_require_finite=True,
            sim_require_nnan=True,
            nc=nc,
        )
        return tuple(outs)

    devices = jax.devices()[:B]
    mesh = Mesh(np.asarray(devices), ("core",))
    spec = jax.sharding.NamedSharding(mesh, PartitionSpec("core"))
    nin = len(in_names)
    sharded = jax.jit(
        bass2jax.shard_map(
            _body,
            mesh=mesh,
            in_specs=(PartitionSpec("core"),) * (nin + len(out_names)),
            out_specs=(PartitionSpec("core"),) * len(out_names),
            check_rep=False,
        ),
        keep_unused=True,
    )

    zeros_dev = [
        jax.device_put(np.zeros((B * s[0], *s[1:]), d), spec)
        for (s, d) in zero_shapes
    ]

    def put(in_maps):
        """Transfer per-core inputs to the devices once; reuse via run_dev."""
        concat_in = [
            np.concatenate([np.asarray(m[name]) for m in in_maps], axis=0)
            for name in in_names
        ]
        return [jax.device_put(a, spec) for a in concat_in]

    def run_dev(dev_in):
        return sharded(*dev_in, *zeros_dev)

    def run(in_maps):
        out_arrs = run_dev(put(in_maps))
        return [
            {
                name: np.asarray(out_arrs[i]).reshape(B, *out_avals[i].shape)[c]
                for i, name in enumerate(out_names)
            }
            for c in range(B)
        ]

    run.put = put
    run.run_dev = run_dev
    _CACHE[key] = run
    return run


def _gating(x, wG, bG):
    """Exact host-side gating: logits -> softmax -> top-2 (matches jax)."""
    xbar = x.astype(np.float64).mean(axis=(2, 3))            # [B, C]
    logits = xbar @ wG[:, :, 0, 0].astype(np.float64).T + bG.astype(np.float64)
    logits -= logits.max(axis=1, keepdims=True)
    p = np.exp(logits)
    probs = p / p.sum(axis=1, keepdims=True)
    idx = np.argsort(-probs, axis=1, kind="stable")[:, :TOP_K]
    gw = np.take_along_axis(probs, idx, axis=1)
    return gw.astype(np.float32), idx


def _make_in_maps(x, wA, bA, wB, bB, gw, idx):
    in_maps = []
    for b in range(B):
        e0, e1 = int(idx[b, 0]), int(idx[b, 1])
        g0, g1 = float(gw[b, 0]), float(gw[b, 1])
        # conv3x3 lhsT: [tap, c-chunk, c%128, m] with m = expert*64 + rank
        wa2 = wA[[e0, e1]]                                   # [2, R, C, 3, 3]
        t = wa2.transpose(3, 4, 2, 0, 1).reshape(9, C, 128)   # [o, c, m]
        wa_host = np.ascontiguousarray(
            t.reshape(9, NCH, 128, 128).transpose(2, 0, 1, 3).reshape(128, -1)
        ).astype(np.float32)
        # 1x1 conv lhsT with gates folded in: rows = [g0*wB[e0].T ; g1*wB[e1].T]
        wb_host = np.concatenate(
            [g0 * wB[e0, :, :, 0, 0].T, g1 * wB[e1, :, :, 0, 0].T], axis=0
        ).astype(np.float32)
        wb_host = np.ascontiguousarray(wb_host)
        cc = (g0 * bB[e0] + g1 * bB[e1]).astype(np.float32)   # [C]
        cc_host = np.ascontiguousarray(cc.reshape(NCH, 128).T)
        ba_host = np.concatenate([bA[e0], bA[e1]]).astype(np.float32).reshape(128, 1)
        in_maps.append(
            {
                "x": np.pad(x[b], ((0, 0), (1, 1), (1, 1))),
                "wa": wa_host,
                "wb": wb_host,
                "cc": np.ascontiguousarray(cc_host),
                "ba": np.ascontiguousarray(ba_host),
            }
        )
    return in_maps


def kernel(x, wA, bA, wB, bB, wG, bG):
    x = np.asarray(x, dtype=np.float32)
    wA = np.asarray(wA, dtype=np.float32)
    bA = np.asarray(bA, dtype=np.float32)
    wB = np.asarray(wB, dtype=np.float32)
    bB = np.asarray(bB, dtype=np.float32)
    wG = np.asarray(wG, dtype=np.float32)
    bG = np.asarray(bG, dtype=np.float32)

    gw, idx = _gating(x, wG, bG)
    in_maps = _make_in_maps(x, wA, bA, wB, bB, gw, idx)
    results = _get_runner()(in_maps)
    out = np.stack([results[b]["out"] for b in range(B)], axis=0)
    return out, gw[:, :, None, None, None]


# revision 9
# speedup vs baseline: 31011.4811x; 2.0383x over previous
"""MoE low-rank conv kernel for Trainium2 (8 NeuronCores, expert-dispatch).

Reference computation (B=8, C=512, H=W=64, E=8, R=64, TOP_K=2):
  expert_e(x) = conv1x1(conv3x3(x, wA[e]) + bA[e], wB[e]) + bB[e]
  logits = meanpool(conv1x1(x, wG) + bG);  probs = softmax(logits)
  gw, idx = top_k(probs, 2)
  out[b] = sum_k gw[b,k] * expert_{idx[b,k]}(x[b]) * x[b]
  returns (out, gw[:, :, None, None, None])

Strategy: gating/top-k is tiny -> computed on host exactly. Only the
2 selected experts per sample are computed on device (the other 6 experts
never influence the output). One NeuronCore per batch sample; the two
selected experts are packed side by side into the 128-wide PE array
(conv3x3 lhsT: K=128 input-channel chunk, M = 2 experts x 64 ranks) so all
matmuls run the full 128x128 array. conv3x3 is computed as 9 shifted
accumulating matmuls over a zero-padded SBUF image. The 1x1 conv of both
experts *and* the gate-weighted sum collapse into a single K=128 matmul per
output tile by pre-scaling wB with the gate weights. Biases ride along on
the ScalarE psum->SBUF evacuations. Matmuls use float32r (FP22) which runs
at full PE rate with ~1e-4 relative precision.
"""

import numpy as np
from contextlib import ExitStack

import jax
from jax.sharding import Mesh, PartitionSpec

import concourse.bass as bass
import concourse.tile as tile
from concourse import bacc, mybir

B, C, H, W = 8, 512, 64, 64
E, R, TOP_K = 8, 64, 2
NCH = C // 128          # 4 input/output channel chunks
NBLK = H // 8           # 8 row-blocks of 8 rows (8*64 = 512 cols = 1 psum bank)
F32 = mybir.dt.float32
F32R = mybir.dt.float32r

_CACHE: dict = {}


def _build_nc(repeat=1, loop=1):
    nc = bacc.Bacc("TRN2", target_bir_lowering=False, debug=False)
    x_d = nc.dram_tensor("x", [C, H + 2, W + 2], F32R, kind="ExternalInput").ap()
    wa_d = nc.dram_tensor("wa", [128, 9 * NCH * 128], F32R, kind="ExternalInput").ap()
    wb_d = nc.dram_tensor("wb", [128, C], F32R, kind="ExternalInput").ap()
    cc_d = nc.dram_tensor("cc", [128, NCH], F32, kind="ExternalInput").ap()
    ba_d = nc.dram_tensor("ba", [128, 1], F32, kind="ExternalInput").ap()
    out_d = nc.dram_tensor("out", [C, H, W], F32, kind="ExternalOutput").ap()

    with tile.TileContext(nc) as tc:
        with ExitStack() as ctx:
            singles = ctx.enter_context(tc.tile_pool(name="singles", bufs=1))
            work = ctx.enter_context(tc.tile_pool(name="work", bufs=3))
            psa = ctx.enter_context(tc.tile_pool(name="psa", bufs=2, space="PSUM"))
            psb = ctx.enter_context(tc.tile_pool(name="psb", bufs=3, space="PSUM"))

            # ---- resident inputs ----
            # DMA issue order = arrival order on the serial queue model, so
            # the pieces the first conv block needs are issued first:
            # x rows [0,10) of all chunks, tap-0 weights, bias; the rest of
            # x and the remaining taps stream in block-consumption order.
            wa_t = singles.tile([128, 9 * NCH * 128], F32R)
            xp = singles.tile([128, NCH, H + 2, W + 2], F32R)
            row_groups = [(0, 10)] + [(10 + 8 * i, 18 + 8 * i) for i in range(7)]
            for k in range(NCH):
                r0, r1 = row_groups[0]
                nc.sync.dma_start(
                    xp[:, k, r0:r1, :], x_d[k * 128 : (k + 1) * 128, r0:r1, :]
                )
            nc.sync.dma_start(wa_t[:, 0 : NCH * 128], wa_d[:, 0 : NCH * 128])
            ba_t = singles.tile([128, 1], F32)
            nc.sync.dma_start(ba_t[:], ba_d[:])
            for o in range(1, 9):
                c0, c1 = o * NCH * 128, (o + 1) * NCH * 128
                nc.sync.dma_start(wa_t[:, c0:c1], wa_d[:, c0:c1])
                if o < 8:
                    r0, r1 = row_groups[o]
                    for k in range(NCH):
                        nc.sync.dma_start(
                            xp[:, k, r0:r1, :],
                            x_d[k * 128 : (k + 1) * 128, r0:r1, :],
                        )
            wb_t = singles.tile([128, C], F32R)
            nc.sync.dma_start(wb_t[:], wb_d[:])
            cc_t = singles.tile([128, NCH], F32)
            nc.sync.dma_start(cc_t[:], cc_d[:])

            # conv3x3 outputs (+bias) for both experts: rank r of expert i at
            # partition i*64+r
            hh = singles.tile([128, H * W], F32R)

            def conv_a_block(blk):
                # ---- conv3x3: 9 shifted accumulating matmuls x 4 c-chunks ----
                h0 = blk * 8
                ph = psa.tile([128, 8 * W], F32)
                last = (2, 2, NCH - 1)
                for dy in range(3):
                    for dx in range(3):
                        for k in range(NCH):
                            o = dy * 3 + dx
                            lhsT = wa_t[:, (o * NCH + k) * 128 : (o * NCH + k + 1) * 128]
                            rhs = xp[:, k, h0 + dy : h0 + dy + 8, dx : dx + W]
                            nc.tensor.matmul(
                                ph[:],
                                lhsT,
                                rhs,
                                start=(dy, dx, k) == (0, 0, 0),
                                stop=(dy, dx, k) == last,
                            )
                nc.scalar.activation(
                    out=hh[:, h0 * W : (h0 + 8) * W],
                    in_=ph[:],
                    func=mybir.ActivationFunctionType.Identity,
                    bias=ba_t[:, 0:1],
                )

            def conv_b_block(blk):
                # ---- 1x1 conv + gate-weighted combine + (*x) + store ----
                h0 = blk * 8
                for m in range(NCH):
                    py = psb.tile([128, 8, W], F32)
                    nc.tensor.matmul(
                        py[:],
                        wb_t[:, m * 128 : (m + 1) * 128],
                        hh[:, h0 * W : (h0 + 8) * W],
                        start=True,
                        stop=True,
                    )
                    yb = work.tile([128, 8, W], F32)
                    nc.scalar.activation(
                        out=yb[:],
                        in_=py[:],
                        func=mybir.ActivationFunctionType.Identity,
                        bias=cc_t[:, m : m + 1],
                    )
                    ot = work.tile([128, 8, W], F32)
                    nc.vector.tensor_mul(
                        ot[:], yb[:], xp[:, m, h0 + 1 : h0 + 9, 1 : W + 1].bitcast(F32)
                    )
                    nc.sync.dma_start(
                        out_d[m * 128 : (m + 1) * 128, h0 : h0 + 8, :], ot[:]
                    )

            def body():
                # conv B of block k-1 is emitted between conv A blocks k and
                # k+1 so the PE stream never waits on ACT/DVE consumers.
                for blk in range(NBLK):
                    conv_a_block(blk)
                    if blk >= 1:
                        conv_b_block(blk - 1)
                conv_b_block(NBLK - 1)

            # repeat/loop>1 build benchmarking NEFFs that replay the body
            # (python-unrolled and/or an on-device For_i loop).
            if loop > 1:
                with tc.For_i(0, loop, 1):
                    for _rep in range(repeat):
                        body()
            else:
                for _rep in range(repeat):
                    body()
    nc.compile()
    return nc


def _get_runner(repeat=1, loop=1):
    """Compile once; return a function(in_maps) -> list of per-core out dicts."""
    key = ("runner", repeat, loop)
    if key in _CACHE:
        return _CACHE[key]

    from concourse import bass2jax

    nc = _build_nc(repeat, loop)
    bass2jax.install_neuronx_cc_hook()

    partition_name = nc.partition_id_tensor.name if nc.partition_id_tensor else None
    in_names, out_names, out_avals, zero_shapes = [], [], [], []
    for alloc in nc.m.functions[0].allocations:
        if not isinstance(alloc, mybir.MemoryLocationSet):
            continue
        name = alloc.memorylocations[0].name
        if alloc.kind == "ExternalInput":
            if name != partition_name:
                in_names.append(name)
        elif alloc.kind == "ExternalOutput":
            out_names.append(name)
            shape = tuple(alloc.tensor_shape)
            dtype = mybir.dt.np(alloc.dtype)
            out_avals.append(jax.core.ShapedArray(shape, dtype))
            zero_shapes.append((shape, dtype))
    n_params = len(in_names)
    n_outs = len(out_names)
    all_names = in_names + out_names
    if partition_name is not None:
        all_names = all_names + [partition_name]
    del n_outs

    def _body(*args):
        operands = list(args)
        if partition_name is not None:
            operands.append(bass2jax.partition_id_tensor())
        outs = bass2jax._bass_exec_p.bind(
            *operands,
            out_avals=tuple(out_avals),
            in_names=tuple(all_names),
            out_names=tuple(out_names),
            lowering_input_output_aliases=(),
            sim_require_finite=True,
            sim_require_nnan=True,
            nc=nc,
        )
        return tuple(outs)

    devices = jax.devices()[:B]
    mesh = Mesh(np.asarray(devices), ("core",))
    spec = jax.sharding.NamedSharding(mesh, PartitionSpec("core"))
    nin = len(in_names)
    sharded = jax.jit(
        bass2jax.shard_map(
            _body,
            mesh=mesh,
            in_specs=(PartitionSpec("core"),) * (nin + len(out_names)),
            out_specs=(PartitionSpec("core"),) * len(out_names),
            check_rep=False,
        ),
        keep_unused=True,
    )

    zeros_dev = [
        jax.device_put(np.zeros((B * s[0], *s[1:]), d), spec)
        for (s, d) in zero_shapes
    ]

    def put(in_maps):
        """Transfer per-core inputs to the devices once; reuse via run_dev."""
        concat_in = [
            np.concatenate([np.asarray(m[name]) for m in in_maps], axis=0)
            for name in in_names
        ]
        return [jax.device_put(a, spec) for a in concat_in]

    def run_dev(dev_in):
        return sharded(*dev_in, *zeros_dev)

    def run(in_maps):
        out_arrs = run_dev(put(in_maps))
        return [
            {
                name: np.asarray(out_arrs[i]).reshape(B, *out_avals[i].shape)[c]
                for i, name in enumerate(out_names)
            }
            for c in range(B)
        ]

    run.put = put
    run.run_dev = run_dev
    _CACHE[key] = run
    return run


def _gating(x, wG, bG):
    """Exact host-side gating: logits -> softmax -> top-2 (matches jax)."""
    xbar = x.astype(np.float64).mean(axis=(2, 3))            # [B, C]
    logits = xbar @ wG[:, :, 0, 0].astype(np.float64).T + bG.astype(np.float64)
    logits -= logits.max(axis=1, keepdims=True)
    p = np.exp(logits)
    probs = p / p.sum(axis=1, keepdims=True)
    idx = np.argsort(-probs, axis=1, kind="stable")[:, :TOP_K]
    gw = np.take_along_axis(probs, idx, axis=1)
    return gw.astype(np.float32), idx


def _make_in_maps(x, wA, bA, wB, bB, gw, idx):
    in_maps = []
    for b in range(B):
        e0, e1 = int(idx[b, 0]), int(idx[b, 1])
        g0, g1 = float(gw[b, 0]), float(gw[b, 1])
        # conv3x3 lhsT: [tap, c-chunk, c%128, m] with m = expert*64 + rank
        wa2 = wA[[e0, e1]]                                   # [2, R, C, 3, 3]
        t = wa2.transpose(3, 4, 2, 0, 1).reshape(9, C, 128)   # [o, c, m]
        wa_host = np.ascontiguousarray(
            t.reshape(9, NCH, 128, 128).transpose(2, 0, 1, 3).reshape(128, -1)
        ).astype(np.float32)
        # 1x1 conv lhsT with gates folded in: rows = [g0*wB[e0].T ; g1*wB[e1].T]
        wb_host = np.concatenate(
            [g0 * wB[e0, :, :, 0, 0].T, g1 * wB[e1, :, :, 0, 0].T], axis=0
        ).astype(np.float32)
        wb_host = np.ascontiguousarray(wb_host)
        cc = (g0 * bB[e0] + g1 * bB[e1]).astype(np.float32)   # [C]
        cc_host = np.ascontiguousarray(cc.reshape(NCH, 128).T)
        ba_host = np.concatenate([bA[e0], bA[e1]]).astype(np.float32).reshape(128, 1)
        in_maps.append(
            {
                "x": np.pad(x[b], ((0, 0), (1, 1), (1, 1))),
                "wa": wa_host,
                "wb": wb_host,
                "cc": np.ascontiguousarray(cc_host),
                "ba": np.ascontiguousarray(ba_host),
            }
        )
    return in_maps


def kernel(x, wA, bA, wB, bB, wG, bG):
    x = np.asarray(x, dtype=np.float32)
    wA = np.asarray(wA, dtype=np.float32)
    bA = np.asarray(bA, dtype=np.float32)
    wB = np.asarray(wB, dtype=np.float32)
    bB = np.asarray(bB, dtype=np.float32)
    wG = np.asarray(wG, dtype=np.float32)
    bG = np.asarray(bG, dtype=np.float32)

    gw, idx = _gating(x, wG, bG)
    in_maps = _make_in_maps(x, wA, bA, wB, bB, gw, idx)
    results = _get_runner()(in_maps)
    out = np.stack([results[b]["out"] for b in range(B)], axis=0)
    return out, gw[:, :, None, None, None]
